# revision 1
# baseline (speedup 1.0000x reference)
"""Sliding-window GQA attention (B=2,T=2048,D=2048,N=8,K=4,H=256,W=1024) on 8 trn2 cores.

Sharding: batch over 2 (fsdp) x heads over 4 (tp). Core (b, tp) computes 2 q heads /
1 kv head for batch b; partial [T, D] outputs are summed over tp on the host.

Per-core device pipeline (all matmuls in float32r — full PE rate, ~1e-4 rel rounding):
  A: qT/kT = W^T x^T (head-dim on partitions) and v (natural layout), streaming x^T
     by 512-token quarters; fused RMS-norm (sum-of-squares via all-ones matmul ->
     replicated rows, rsqrt via ACT spline) + RoPE (host sin/cos tables) out of PSUM.
  B: per 256-token query pair: logits^T = kT^T qT per 128-key block (only window
     blocks), exp on ACT (no max-subtraction: |logit| <= 16), triangular masks on
     DVE/GpSimd, denominator + P^T V via PE accumulation, divide via rsqrt^2 on DVE.
  C: out = pvT^T o_w accumulated over local heads, emitted lagged one pair behind
     phase B so its matmuls fill exp-wait windows; copies alternate DVE/ACT.
"""
import os

import numpy as np

import concourse.bacc as bacc
import concourse.mybir as mybir
from concourse.tile import TileContext
from concourse.bass_utils import run_bass_kernel_spmd

try:  # pragma: no cover - profiling hook is optional
    from antenv.axon_hooks import get_axon_ntff_profile_hook  # noqa: F401
except ImportError:
    # No NTFF hook available in this environment: make sure a stray
    # BASS_TRACE=1 cannot break execution.
    os.environ.setdefault("BASS_NEVER_TRACE", "1")


F32 = mybir.dt.float32
F32R = mybir.dt.float32r
AF = mybir.ActivationFunctionType
OP = mybir.AluOpType

B, T, D = 2, 2048, 2048
N, KV, H = 8, 4, 256
WINDOW = 1024
BASE_FREQ = 10000.0
EPS = 1e-6
NB = T // 128          # 16 token blocks
NQ = 4                 # t quarters for projections (512 each)
NPAIR = 8              # query-block pairs (256 tokens each)


def _mask_idx(i, j):
    if j == i + 1:
        return 3
    if j == i:
        return 2
    if j == i - 7:
        return 1
    if j == i - 8:
        return 0
    return None


def _jlist(i):
    return list(range(max(0, i - 8), i + 2))


def _build():
    nc = bacc.Bacc(None)

    xT = nc.dram_tensor("xT", [D, T], F32R, kind="ExternalInput")
    qw = nc.dram_tensor("qw", [2, D, H], F32R, kind="ExternalInput")
    kw = nc.dram_tensor("kw", [D, H], F32R, kind="ExternalInput")
    vw = nc.dram_tensor("vw", [D, H], F32R, kind="ExternalInput")
    ow = nc.dram_tensor("ow", [2, H, D], F32R, kind="ExternalInput")
    cosT = nc.dram_tensor("cosT", [128, T], F32, kind="ExternalInput")
    sinT = nc.dram_tensor("sinT", [128, T], F32, kind="ExternalInput")
    masks = nc.dram_tensor("masks", [128, 4, 256], F32R, kind="ExternalInput")
    scs = nc.dram_tensor("scs", [128, 2, 2], F32, kind="ExternalInput")  # (1+scale)[q/k][hh]
    out = nc.dram_tensor("out", [T, D], F32, kind="ExternalOutput")

    with TileContext(nc) as tc:
        with tc.tile_pool(name="pers", bufs=1) as pers:
            kT_sb = pers.tile([128, 2, T], F32R)
            v_sb = pers.tile([128, NB, H], F32R)
            qT_sb = pers.tile([128, 2, 2, T], F32R)
            scs_sb = pers.tile([128, 2, 2], F32)
            ones32 = pers.tile([128, 128], F32)
            ones = pers.tile([128, 128], F32R)
            bias_q = pers.tile([128, 1], F32)
            bias_k = pers.tile([128, 1], F32)
            bias_z = pers.tile([128, 1], F32)

            nc.sync.dma_start(out=scs_sb, in_=scs[:, :, :])
            nc.vector.memset(ones32, 1.0)
            nc.vector.tensor_copy(ones, ones32)
            nc.vector.memset(bias_q, float(H * EPS))
            nc.vector.memset(bias_k, EPS)
            nc.vector.memset(bias_z, 0.0)

            # ---------------- Phase A: all projections + rms + rope ----------------
            with tc.tile_pool(name="wts", bufs=1) as wts, \
                 tc.tile_pool(name="xs", bufs=20) as xs, \
                 tc.tile_pool(name="ropep", bufs=1) as ropep, \
                 tc.tile_pool(name="psA", bufs=1, space="PSUM") as psum:

                # per-chunk weight tiles so consumers wait on single DMAs.
                # Weights ride the second HWDGE ring (ACT) so they don't queue
                # ahead of the x^T stream on the SP ring.
                kw_t, vw_t, qw_t = {}, {}, {}
                for d in range(16):
                    ds = slice(128 * d, 128 * (d + 1))
                    kw_t[d] = wts.tile([128, H], F32R, tag=f"kw{d}", name=f"kw{d}")
                    nc.scalar.dma_start(out=kw_t[d], in_=kw[ds, :])
                    vw_t[d] = wts.tile([128, H], F32R, tag=f"vw{d}", name=f"vw{d}")
                    nc.scalar.dma_start(out=vw_t[d], in_=vw[ds, :])

                def rope_emit(p0, p1, dst, kind, cs_t, ss_t):
                    # p0/p1: [128, 512] psum (raw proj h-halves); dst: [128, 2, 512] f32r view
                    sq0 = ropep.tile([128, 512], F32R, tag="sq0", bufs=2)
                    sq1 = ropep.tile([128, 512], F32R, tag="sq1", bufs=2)
                    nc.scalar.activation(sq0, p0, AF.Square)
                    nc.scalar.activation(sq1, p1, AF.Square)
                    pss = psum.tile([128, 512], F32, tag="pss", bufs=1)
                    nc.tensor.matmul(pss, ones, sq0, start=True, stop=False)
                    nc.tensor.matmul(pss, ones, sq1, start=False, stop=True)
                    rs = ropep.tile([128, 512], F32, tag="rs", bufs=2)
                    if kind == "q":
                        # 1/16 * rsqrt(ss/256 + eps) == 1/sqrt(ss + 256*eps)
                        nc.scalar.activation(rs, pss, AF.Abs_reciprocal_sqrt,
                                             scale=1.0, bias=bias_q)
                    else:
                        nc.scalar.activation(rs, pss, AF.Abs_reciprocal_sqrt,
                                             scale=1.0 / H, bias=bias_k)
                    cs = ropep.tile([128, 512], F32, tag="cs", bufs=2)
                    ss = ropep.tile([128, 512], F32, tag="ss", bufs=2)
                    nc.vector.tensor_tensor(cs, cs_t, rs, OP.mult)
                    nc.vector.tensor_tensor(ss, ss_t, rs, OP.mult)
                    ki = 0 if kind == "q" else 1
                    s0 = scs_sb[:, ki, 0:1]
                    s1 = scs_sb[:, ki, 1:2]
                    t0 = ropep.tile([128, 512], F32, tag="t0", bufs=2)
                    t1 = ropep.tile([128, 512], F32, tag="t1", bufs=2)
                    nc.vector.scalar_tensor_tensor(t0, p0, s0, cs, OP.mult, OP.mult)
                    nc.vector.scalar_tensor_tensor(t1, p1, s1, ss, OP.mult, OP.mult)
                    nc.vector.tensor_tensor(dst[:, 0, :], t0, t1, OP.subtract)
                    t2 = ropep.tile([128, 512], F32, tag="t0", bufs=2)
                    t3 = ropep.tile([128, 512], F32, tag="t1", bufs=2)
                    nc.vector.scalar_tensor_tensor(t2, p1, s1, cs, OP.mult, OP.mult)
                    nc.vector.scalar_tensor_tensor(t3, p0, s0, ss, OP.mult, OP.mult)
                    nc.vector.tensor_tensor(dst[:, 1, :], t2, t3, OP.add)

                for qt in range(NQ):
                    tq = slice(512 * qt, 512 * (qt + 1))
                    xts = []
                    for d in range(16):
                        xt = xs.tile([128, 512], F32R, tag="xt")
                        nc.sync.dma_start(out=xt, in_=xT[slice(128 * d, 128 * (d + 1)), tq])
                        xts.append(xt)
                    if qt == 0:
                        # q weights load while the k/v projections run
                        for d in range(16):
                            ds = slice(128 * d, 128 * (d + 1))
                            for nl in range(2):
                                qw_t[(nl, d)] = wts.tile([128, H], F32R, tag=f"qw{nl}_{d}", name=f"qw{nl}_{d}")
                                nc.scalar.dma_start(out=qw_t[(nl, d)], in_=qw[nl, ds, :])
                    cs_t = ropep.tile([128, 512], F32, tag="cst", bufs=2)
                    ss_t = ropep.tile([128, 512], F32, tag="sst", bufs=2)
                    nc.scalar.dma_start(out=cs_t, in_=cosT[:, tq])
                    nc.scalar.dma_start(out=ss_t, in_=sinT[:, tq])

                    # k h-halves interleaved per d-chunk (separate PSUM banks) so
                    # PE consumption keeps pace with the x^T DMA stream early on.
                    # NB: accumulation groups must NOT interleave within one bank
                    # (a group's start clears the whole bank's has_written bits).
                    pk = [psum.tile([128, 512], F32, tag="pq", bufs=6, name=f"pk{qt}_{hh}")
                          for hh in range(2)]
                    for d in range(16):
                        nc.tensor.matmul(pk[0], kw_t[d][:, 0:128], xts[d],
                                         start=(d == 0), stop=(d == 15))
                        nc.tensor.matmul(pk[1], kw_t[d][:, 128:256], xts[d],
                                         start=(d == 0), stop=(d == 15))
                    rope_emit(pk[0], pk[1], kT_sb[:, :, tq], "k", cs_t, ss_t)
                    # v (natural layout): sequential accumulation groups per bank
                    for half in range(2):
                        p = psum.tile([128, 2, H], F32, tag="pva", bufs=1, name=f"pv{qt}_{half}")
                        tc0 = 4 * qt + 2 * half
                        for sub in range(2):
                            tl = slice(128 * (2 * half + sub), 128 * (2 * half + sub) + 128)
                            for d in range(16):
                                nc.tensor.matmul(p[:, sub, :], xts[d][:, tl], vw_t[d],
                                                 start=(d == 0), stop=(d == 15))
                        nc.vector.tensor_copy(v_sb[:, tc0:tc0 + 2, :], p)
                    # q heads
                    for nl in range(2):
                        ps = []
                        for hh in range(2):
                            p = psum.tile([128, 512], F32, tag="pq", bufs=6)
                            hs = slice(128 * hh, 128 * (hh + 1))
                            for d in range(16):
                                nc.tensor.matmul(p, qw_t[(nl, d)][:, hs], xts[d],
                                                 start=(d == 0), stop=(d == 15))
                            ps.append(p)
                        rope_emit(ps[0], ps[1], qT_sb[:, nl, :, tq], "q", cs_t, ss_t)

            # ---------------- Phases B + C ----------------
            with tc.tile_pool(name="pers2", bufs=1) as pers2:
                pvT_sb = pers2.tile([128, 2, 2, T], F32R)
                ow_sb = pers2.tile([128, 2, 2, D], F32R)
                masks_sb = pers2.tile([128, 4, 256], F32R)
                nc.scalar.dma_start(out=masks_sb, in_=masks[:, :, :])
                for nl in range(2):
                    for hh in range(2):
                        nc.scalar.dma_start(out=ow_sb[:, nl, hh, :],
                                            in_=ow[nl, slice(128 * hh, 128 * (hh + 1)), :])

                with tc.tile_pool(name="expt", bufs=14) as expt, \
                     tc.tile_pool(name="bw", bufs=4) as bw, \
                     tc.tile_pool(name="oc", bufs=3) as oc, \
                     tc.tile_pool(name="psB", bufs=1, space="PSUM") as psumB:

                    def emit_logits_exp(pi):
                        i = 2 * pi
                        tqs = slice(256 * pi, 256 * (pi + 1))
                        js = _jlist(i)
                        ets = {}
                        for nl in range(2):
                            for k in range(0, len(js), 2):
                                jp = js[k:k + 2]
                                lp = psumB.tile([128, 2, 256], F32, tag="lp", bufs=4)
                                for x2, j in enumerate(jp):
                                    sj = slice(128 * j, 128 * (j + 1))
                                    nc.tensor.matmul(lp[:, x2, :], kT_sb[:, 0, sj],
                                                     qT_sb[:, nl, 0, tqs],
                                                     start=True, stop=False)
                                    nc.tensor.matmul(lp[:, x2, :], kT_sb[:, 1, sj],
                                                     qT_sb[:, nl, 1, tqs],
                                                     start=False, stop=True)
                                et = expt.tile([128, 2, 256], F32R, tag="et")
                                nc.scalar.activation(et, lp, AF.Exp)
                                for x2, j in enumerate(jp):
                                    mi = _mask_idx(i, j)
                                    if mi is not None:
                                        eng = nc.vector if (j % 2 == 0) else nc.gpsimd
                                        eng.tensor_tensor(et[:, x2, :], et[:, x2, :],
                                                          masks_sb[:, mi, :], OP.mult)
                                    ets[(nl, j)] = et[:, x2, :]
                        return ets

                    def emit_tail(pi, ets):
                        i = 2 * pi
                        tqs = slice(256 * pi, 256 * (pi + 1))
                        js = _jlist(i)
                        for nl in range(2):
                            pd = psumB.tile([128, 256], F32, tag="pd", bufs=1)
                            for idx, j in enumerate(js):
                                nc.tensor.matmul(pd, ones, ets[(nl, j)],
                                                 start=(idx == 0), stop=(idx == len(js) - 1))
                            # 1/den via rsqrt(den)^2 (cheap ACT spline + two DVE mults)
                            r1 = bw.tile([128, 256], F32, tag="r1")
                            nc.scalar.activation(r1, pd, AF.Abs_reciprocal_sqrt,
                                                 scale=1.0, bias=bias_z)
                            for hh in range(2):
                                pv = psumB.tile([128, 256], F32, tag="pvb", bufs=3)
                                hs = slice(128 * hh, 128 * (hh + 1))
                                for idx, j in enumerate(js):
                                    nc.tensor.matmul(pv, v_sb[:, j, hs], ets[(nl, j)],
                                                     start=(idx == 0), stop=(idx == len(js) - 1))
                                tmp = bw.tile([128, 256], F32, tag="tmp")
                                nc.vector.tensor_tensor(tmp, pv, r1, OP.mult)
                                nc.vector.tensor_tensor(pvT_sb[:, nl, hh, tqs], tmp, r1, OP.mult)
                    def emit_oproj(pi):
                        # output projection for this pair's two token blocks
                        for tb in (2 * pi, 2 * pi + 1):
                            ts_ = slice(128 * tb, 128 * (tb + 1))
                            for dt in range(4):
                                dsl = slice(512 * dt, 512 * (dt + 1))
                                po = psumB.tile([128, 512], F32, tag="pvb", bufs=3)
                                step = 0
                                for nl in range(2):
                                    for hh in range(2):
                                        nc.tensor.matmul(po, pvT_sb[:, nl, hh, ts_],
                                                         ow_sb[:, nl, hh, dsl],
                                                         start=(step == 0), stop=(step == 3))
                                        step += 1
                                od = oc.tile([128, 512], F32, tag="od", bufs=6)
                                if (tb * 4 + dt) % 2 == 0:
                                    nc.vector.tensor_copy(od, po)
                                else:
                                    nc.scalar.copy(od, po)
                                nc.sync.dma_start(out=out[ts_, dsl], in_=od)

                    # o-proj lags one pair behind: its matmuls fill exp-wait
                    # windows and give the ow DMA time to land after phase A.
                    for pi in range(NPAIR):
                        emit_tail(pi, emit_logits_exp(pi))
                        if pi > 0:
                            emit_oproj(pi - 1)
                    emit_oproj(NPAIR - 1)

    nc.compile()
    return nc


_prog = None
last_results = None


def kernel(x, positions, q_w, k_w, v_w, o_w, q_norm_scale, k_norm_scale):
    global _prog, last_results
    x = np.asarray(x); positions = np.asarray(positions)
    q_w = np.asarray(q_w); k_w = np.asarray(k_w); v_w = np.asarray(v_w); o_w = np.asarray(o_w)
    q_norm_scale = np.asarray(q_norm_scale); k_norm_scale = np.asarray(k_norm_scale)

    if _prog is None:
        _prog = _build()
    nc = _prog

    # host-side constants
    j = np.arange(H // 2, dtype=np.float32)
    timescale = (BASE_FREQ ** (2.0 / H * j)).astype(np.float32)

    c = np.arange(128)[:, None]
    r = np.arange(128)[None, :]
    up = (c <= r).astype(np.float32)
    lo = (c > r).astype(np.float32)
    one_b = np.ones((128, 128), np.float32)
    zero_b = np.zeros((128, 128), np.float32)
    masks_np = np.stack([
        np.concatenate([lo, zero_b], 1),
        np.concatenate([one_b, lo], 1),
        np.concatenate([up, one_b], 1),
        np.concatenate([zero_b, up], 1),
    ], axis=0).transpose(1, 0, 2).copy()  # [128, 4, 256]

    scs_np = np.empty((128, 2, 2), np.float32)
    scs_np[:, 0, 0] = 1.0 + q_norm_scale[:128]
    scs_np[:, 0, 1] = 1.0 + q_norm_scale[128:]
    scs_np[:, 1, 0] = 1.0 + k_norm_scale[:128]
    scs_np[:, 1, 1] = 1.0 + k_norm_scale[128:]

    in_maps = []
    for core in range(8):
        b, tp = core // 4, core % 4
        sinu = positions[b].astype(np.float32)[:, None] / timescale[None, :]  # [T, 128]
        in_maps.append({
            "xT": np.ascontiguousarray(x[b].T).astype(np.float32),
            "qw": np.ascontiguousarray(q_w[2 * tp: 2 * tp + 2]).astype(np.float32),
            "kw": np.ascontiguousarray(k_w[tp]).astype(np.float32),
            "vw": np.ascontiguousarray(v_w[tp]).astype(np.float32),
            "ow": np.ascontiguousarray(o_w[2 * tp: 2 * tp + 2]).astype(np.float32),
            "cosT": np.ascontiguousarray(np.cos(sinu).T).astype(np.float32),
            "sinT": np.ascontiguousarray(np.sin(sinu).T).astype(np.float32),
            "masks": masks_np,
            "scs": scs_np,
        })

    res = run_bass_kernel_spmd(nc, in_maps, core_ids=list(range(8)))
    last_results = res

    out = np.zeros((B, T, D), np.float32)
    for core in range(8):
        out[core // 4] += res.results[core]["out"]
    return out



# revision 3
# speedup vs baseline: 1.2406x; 1.2406x over previous
"""Sliding-window GQA attention (B=2,T=2048,D=2048,N=8,K=4,H=256,W=1024) on 8 trn2 cores.

Sharding: batch over 2 (fsdp) x heads over 4 (tp). Core (b, tp) computes 2 q heads /
1 kv head for batch b; partial [T, D] outputs are summed over tp on the host.

v2 (bf16): all matmul operands bf16 (fp32 PSUM accumulation) — same PE stream
rate as float32r but FWL halves weight-load time (phase B was LDW-bound), DMA
bytes halve, and SBUF pressure drops. Activation-table thrash eliminated:
phase A ACT = Square+Rsqrt only, phase B ACT = Exp only (1/den moved to the
DVE reciprocal_approx_fast custom op, output copies all on DVE). Host packs
weights in SBUF layout so each weight tensor is one batched DMA. Half-masked
edge key-blocks (j=i+1, j=i-8) only compute their valid 128-query half.

Per-core device pipeline:
  A: qT/kT = W^T x^T (head-dim on partitions) and v (natural layout), streaming
     x^T by 512-token quarters; fused RMS-norm + RoPE out of PSUM.
  B: per 256-token query pair: logits^T = kT^T qT per 128-key block (window
     blocks only), exp on ACT (no max-subtraction: |logit| <= ~6), triangular
     masks on DVE, denominator + P^T V via PE accumulation, divide via
     DVE approx-reciprocal.
  C: out = pvT^T o_w accumulated over local heads, emitted lagged one pair
     behind phase B so its matmuls fill exp-wait windows.
"""
import os

import numpy as np
import ml_dtypes

import concourse.bacc as bacc
import concourse.mybir as mybir
from concourse.tile import TileContext
from concourse.bass_utils import run_bass_kernel_spmd

try:  # pragma: no cover - profiling hook is optional
    from antenv.axon_hooks import get_axon_ntff_profile_hook  # noqa: F401
except ImportError:
    os.environ.setdefault("BASS_NEVER_TRACE", "1")


F32 = mybir.dt.float32
BF = mybir.dt.bfloat16
AF = mybir.ActivationFunctionType
OP = mybir.AluOpType

B, T, D = 2, 2048, 2048
N, KV, H = 8, 4, 256
WINDOW = 1024
BASE_FREQ = 10000.0
EPS = 1e-6
NB = T // 128          # 16 token blocks
NQ = 4                 # t quarters for projections (512 each)
NPAIR = 8              # query-block pairs (256 tokens each)


def _jlist(i):
    return list(range(max(0, i - 8), i + 2))


def _build():
    nc = bacc.Bacc(None)

    xT = nc.dram_tensor("xT", [D, T], BF, kind="ExternalInput")
    qw = nc.dram_tensor("qw", [128, 2, 16, 256], BF, kind="ExternalInput")
    kw = nc.dram_tensor("kw", [128, 16, 256], BF, kind="ExternalInput")
    vw = nc.dram_tensor("vw", [128, 16, 256], BF, kind="ExternalInput")
    ow = nc.dram_tensor("ow", [128, 2, 2, D], BF, kind="ExternalInput")
    cosT = nc.dram_tensor("cosT", [128, T], F32, kind="ExternalInput")
    sinT = nc.dram_tensor("sinT", [128, T], F32, kind="ExternalInput")
    masks = nc.dram_tensor("masks", [128, 2, 128], BF, kind="ExternalInput")
    scs = nc.dram_tensor("scs", [128, 2, 2], F32, kind="ExternalInput")  # (1+scale)[q/k][hh]
    out = nc.dram_tensor("out", [T, D], BF, kind="ExternalOutput")

    with TileContext(nc) as tc:
        with tc.tile_pool(name="pers", bufs=1) as pers:
            kT_sb = pers.tile([128, 2, T], BF)
            v_sb = pers.tile([128, NB, H], BF)
            qT_sb = pers.tile([128, 2, 2, T], BF)
            kw_sb = pers.tile([128, 16, 256], BF)
            vw_sb = pers.tile([128, 16, 256], BF)
            qw_sb = pers.tile([128, 2, 16, 256], BF)
            cos_sb = pers.tile([128, T], F32)
            sin_sb = pers.tile([128, T], F32)
            scs_sb = pers.tile([128, 2, 2], F32)
            ones32 = pers.tile([128, 128], F32)
            ones = pers.tile([128, 128], BF)
            bias_q = pers.tile([128, 1], F32)
            bias_k = pers.tile([128, 1], F32)

            # batched weight DMAs ride the second HWDGE ring (ACT) so they
            # don't queue ahead of the x^T stream on the SP ring.
            nc.scalar.dma_start(out=kw_sb, in_=kw[:, :, :])
            nc.scalar.dma_start(out=vw_sb, in_=vw[:, :, :])
            nc.scalar.dma_start(out=qw_sb, in_=qw[:, :, :, :])
            nc.scalar.dma_start(out=cos_sb, in_=cosT[:, :])
            nc.scalar.dma_start(out=sin_sb, in_=sinT[:, :])
            nc.scalar.dma_start(out=scs_sb, in_=scs[:, :, :])
            nc.vector.memset(ones32, 1.0)
            nc.vector.tensor_copy(ones, ones32)
            nc.vector.memset(bias_q, float(H * EPS))
            nc.vector.memset(bias_k, EPS)

            # ---------------- Phase A: all projections + rms + rope ----------------
            with tc.tile_pool(name="xs", bufs=24) as xs, \
                 tc.tile_pool(name="ropep", bufs=1) as ropep, \
                 tc.tile_pool(name="psA", bufs=1, space="PSUM") as psum:

                def rope_emit(p0, p1, dst, kind, cs_t, ss_t):
                    # p0/p1: [128, 512] psum (raw proj h-halves); dst: [128, 2, 512] bf16 view
                    sq0 = ropep.tile([128, 512], BF, tag="sq0", bufs=2)
                    sq1 = ropep.tile([128, 512], BF, tag="sq1", bufs=2)
                    nc.scalar.activation(sq0, p0, AF.Square)
                    nc.scalar.activation(sq1, p1, AF.Square)
                    pss = psum.tile([128, 512], F32, tag="pss", bufs=1)
                    nc.tensor.matmul(pss, ones, sq0, start=True, stop=False)
                    nc.tensor.matmul(pss, ones, sq1, start=False, stop=True)
                    rs = ropep.tile([128, 512], F32, tag="rs", bufs=2)
                    if kind == "q":
                        # 1/16 * rsqrt(ss/256 + eps) == 1/sqrt(ss + 256*eps)
                        nc.scalar.activation(rs, pss, AF.Abs_reciprocal_sqrt,
                                             scale=1.0, bias=bias_q)
                    else:
                        nc.scalar.activation(rs, pss, AF.Abs_reciprocal_sqrt,
                                             scale=1.0 / H, bias=bias_k)
                    cs = ropep.tile([128, 512], F32, tag="cs", bufs=2)
                    ss = ropep.tile([128, 512], F32, tag="ss", bufs=2)
                    nc.vector.tensor_tensor(cs, cs_t, rs, OP.mult)
                    nc.vector.tensor_tensor(ss, ss_t, rs, OP.mult)
                    ki = 0 if kind == "q" else 1
                    s0 = scs_sb[:, ki, 0:1]
                    s1 = scs_sb[:, ki, 1:2]
                    t0 = ropep.tile([128, 512], F32, tag="t0", bufs=2)
                    t1 = ropep.tile([128, 512], F32, tag="t1", bufs=2)
                    nc.vector.scalar_tensor_tensor(t0, p0, s0, cs, OP.mult, OP.mult)
                    nc.vector.scalar_tensor_tensor(t1, p1, s1, ss, OP.mult, OP.mult)
                    nc.vector.tensor_tensor(dst[:, 0, :], t0, t1, OP.subtract)
                    t2 = ropep.tile([128, 512], F32, tag="t0", bufs=2)
                    t3 = ropep.tile([128, 512], F32, tag="t1", bufs=2)
                    nc.vector.scalar_tensor_tensor(t2, p1, s1, cs, OP.mult, OP.mult)
                    nc.vector.scalar_tensor_tensor(t3, p0, s0, ss, OP.mult, OP.mult)
                    nc.vector.tensor_tensor(dst[:, 1, :], t2, t3, OP.add)

                for qt in range(NQ):
                    tq = slice(512 * qt, 512 * (qt + 1))
                    xts = []
                    for d in range(16):
                        xt = xs.tile([128, 512], BF, tag="xt")
                        nc.sync.dma_start(out=xt, in_=xT[slice(128 * d, 128 * (d + 1)), tq])
                        xts.append(xt)
                    cs_t = cos_sb[:, tq]
                    ss_t = sin_sb[:, tq]

                    # k h-halves interleaved per d-chunk (separate PSUM banks) so
                    # PE consumption keeps pace with the x^T DMA stream early on.
                    # NB: accumulation groups must NOT interleave within one bank.
                    pk = [psum.tile([128, 512], F32, tag="pq", bufs=6, name=f"pk{qt}_{hh}")
                          for hh in range(2)]
                    for d in range(16):
                        nc.tensor.matmul(pk[0], kw_sb[:, d, 0:128], xts[d],
                                         start=(d == 0), stop=(d == 15))
                        nc.tensor.matmul(pk[1], kw_sb[:, d, 128:256], xts[d],
                                         start=(d == 0), stop=(d == 15))
                    rope_emit(pk[0], pk[1], kT_sb[:, :, tq], "k", cs_t, ss_t)
                    # v (natural layout): sequential accumulation groups per bank
                    for half in range(2):
                        p = psum.tile([128, 2, H], F32, tag="pva", bufs=1, name=f"pv{qt}_{half}")
                        tc0 = 4 * qt + 2 * half
                        for sub in range(2):
                            tl = slice(128 * (2 * half + sub), 128 * (2 * half + sub) + 128)
                            for d in range(16):
                                nc.tensor.matmul(p[:, sub, :], xts[d][:, tl], vw_sb[:, d, :],
                                                 start=(d == 0), stop=(d == 15))
                        nc.vector.tensor_copy(v_sb[:, tc0:tc0 + 2, :], p)
                    # q heads
                    for nl in range(2):
                        ps = []
                        for hh in range(2):
                            p = psum.tile([128, 512], F32, tag="pq", bufs=6)
                            hs = slice(128 * hh, 128 * (hh + 1))
                            for d in range(16):
                                nc.tensor.matmul(p, qw_sb[:, nl, d, hs], xts[d],
                                                 start=(d == 0), stop=(d == 15))
                            ps.append(p)
                        rope_emit(ps[0], ps[1], qT_sb[:, nl, :, tq], "q", cs_t, ss_t)

            # ---------------- Phases B + C ----------------
            with tc.tile_pool(name="pers2", bufs=1) as pers2:
                pvT_sb = pers2.tile([128, 2, 2, T], BF)
                ow_sb = pers2.tile([128, 2, 2, D], BF)
                masks_sb = pers2.tile([128, 2, 128], BF)
                nc.scalar.dma_start(out=masks_sb, in_=masks[:, :, :])
                nc.scalar.dma_start(out=ow_sb, in_=ow[:, :, :, :])

                with tc.tile_pool(name="expt", bufs=14) as expt, \
                     tc.tile_pool(name="bw", bufs=4) as bw, \
                     tc.tile_pool(name="oc", bufs=6) as oc, \
                     tc.tile_pool(name="psB", bufs=1, space="PSUM") as psumB:

                    def emit_logits_exp(pi):
                        i = 2 * pi
                        q_lo = slice(256 * pi, 256 * pi + 128)   # first query half
                        q_hi = slice(256 * pi + 128, 256 * (pi + 1))
                        q_all = slice(256 * pi, 256 * (pi + 1))
                        js = _jlist(i)
                        ets = {}
                        up = masks_sb[:, 0, :]
                        lo = masks_sb[:, 1, :]
                        for nl in range(2):
                            for k in range(0, len(js), 2):
                                jp = js[k:k + 2]
                                lp = psumB.tile([128, 2, 256], F32, tag="lp", bufs=4)
                                et = expt.tile([128, 2, 256], BF, tag="et")
                                for x2, j in enumerate(jp):
                                    sj = slice(128 * j, 128 * (j + 1))
                                    if j == i + 1:      # keys above all of q_lo
                                        lps, qsl = lp[:, x2, 128:256], q_hi
                                        ecols = slice(128, 256)
                                    elif j == i - 8:    # keys in-window only for q_lo
                                        lps, qsl = lp[:, x2, 0:128], q_lo
                                        ecols = slice(0, 128)
                                    else:
                                        lps, qsl = lp[:, x2, :], q_all
                                        ecols = slice(0, 256)
                                    nc.tensor.matmul(lps, kT_sb[:, 0, sj],
                                                     qT_sb[:, nl, 0, qsl],
                                                     start=True, stop=False)
                                    nc.tensor.matmul(lps, kT_sb[:, 1, sj],
                                                     qT_sb[:, nl, 1, qsl],
                                                     start=False, stop=True)
                                    ets[(nl, j)] = (et, x2, ecols)
                                # uncomputed edge halves hold stale psum; their
                                # exp lands in et cols no consumer ever reads.
                                nc.scalar.activation(et, lp, AF.Exp)
                                for x2, j in enumerate(jp):
                                    if j == i + 1:
                                        nc.vector.tensor_tensor(
                                            et[:, x2, 128:256], et[:, x2, 128:256], up, OP.mult)
                                    elif j == i:
                                        nc.vector.tensor_tensor(
                                            et[:, x2, 0:128], et[:, x2, 0:128], up, OP.mult)
                                    elif j == i - 7:
                                        nc.vector.tensor_tensor(
                                            et[:, x2, 128:256], et[:, x2, 128:256], lo, OP.mult)
                                    elif j == i - 8:
                                        nc.vector.tensor_tensor(
                                            et[:, x2, 0:128], et[:, x2, 0:128], lo, OP.mult)
                        return ets

                    def emit_tail(pi, ets):
                        i = 2 * pi
                        tqs = slice(256 * pi, 256 * (pi + 1))
                        js = _jlist(i)
                        for nl in range(2):
                            pd = psumB.tile([128, 256], F32, tag="pd", bufs=1)
                            for idx, j in enumerate(js):
                                et, x2, ecols = ets[(nl, j)]
                                nc.tensor.matmul(pd[:, ecols], ones, et[:, x2, ecols],
                                                 start=(idx == 0), stop=(idx == len(js) - 1))
                            rc = bw.tile([128, 256], F32, tag="rc")
                            nc.vector.reciprocal_approx_fast(rc, pd)
                            for hh in range(2):
                                pv = psumB.tile([128, 256], F32, tag="pvb", bufs=3)
                                hs = slice(128 * hh, 128 * (hh + 1))
                                for idx, j in enumerate(js):
                                    et, x2, ecols = ets[(nl, j)]
                                    nc.tensor.matmul(pv[:, ecols], v_sb[:, j, hs],
                                                     et[:, x2, ecols],
                                                     start=(idx == 0), stop=(idx == len(js) - 1))
                                nc.vector.tensor_tensor(pvT_sb[:, nl, hh, tqs], pv, rc, OP.mult)

                    def emit_oproj(pi):
                        # output projection for this pair's two token blocks
                        for tb in (2 * pi, 2 * pi + 1):
                            ts_ = slice(128 * tb, 128 * (tb + 1))
                            for dt in range(4):
                                dsl = slice(512 * dt, 512 * (dt + 1))
                                po = psumB.tile([128, 512], F32, tag="pvb", bufs=3)
                                step = 0
                                for nl in range(2):
                                    for hh in range(2):
                                        nc.tensor.matmul(po, pvT_sb[:, nl, hh, ts_],
                                                         ow_sb[:, nl, hh, dsl],
                                                         start=(step == 0), stop=(step == 3))
                                        step += 1
                                od = oc.tile([128, 512], BF, tag="od", bufs=6)
                                nc.vector.tensor_copy(od, po)
                                nc.sync.dma_start(out=out[ts_, dsl], in_=od)

                    # o-proj lags one pair behind: its matmuls fill exp-wait
                    # windows and give the ow DMA time to land after phase A.
                    for pi in range(NPAIR):
                        emit_tail(pi, emit_logits_exp(pi))
                        if pi > 0:
                            emit_oproj(pi - 1)
                    emit_oproj(NPAIR - 1)

    nc.compile()
    return nc


_prog = None
last_results = None


def kernel(x, positions, q_w, k_w, v_w, o_w, q_norm_scale, k_norm_scale):
    global _prog, last_results
    x = np.asarray(x); positions = np.asarray(positions)
    q_w = np.asarray(q_w); k_w = np.asarray(k_w); v_w = np.asarray(v_w); o_w = np.asarray(o_w)
    q_norm_scale = np.asarray(q_norm_scale); k_norm_scale = np.asarray(k_norm_scale)

    if _prog is None:
        _prog = _build()
    nc = _prog

    bf = ml_dtypes.bfloat16

    # host-side constants
    j = np.arange(H // 2, dtype=np.float32)
    timescale = (BASE_FREQ ** (2.0 / H * j)).astype(np.float32)

    c = np.arange(128)[:, None]   # key within block (partition)
    r = np.arange(128)[None, :]   # query within block (column)
    up = (c <= r).astype(np.float32)
    lo = (c > r).astype(np.float32)
    masks_np = np.stack([up, lo], axis=1).astype(bf)  # [128, 2, 128]

    scs_np = np.empty((128, 2, 2), np.float32)
    scs_np[:, 0, 0] = 1.0 + q_norm_scale[:128]
    scs_np[:, 0, 1] = 1.0 + q_norm_scale[128:]
    scs_np[:, 1, 0] = 1.0 + k_norm_scale[:128]
    scs_np[:, 1, 1] = 1.0 + k_norm_scale[128:]

    in_maps = []
    for core in range(8):
        b, tp = core // 4, core % 4
        sinu = positions[b].astype(np.float32)[:, None] / timescale[None, :]  # [T, 128]
        qw_h = np.ascontiguousarray(
            q_w[2 * tp:2 * tp + 2].reshape(2, 16, 128, H).transpose(2, 0, 1, 3)).astype(bf)
        kw_h = np.ascontiguousarray(
            k_w[tp].reshape(16, 128, H).transpose(1, 0, 2)).astype(bf)
        vw_h = np.ascontiguousarray(
            v_w[tp].reshape(16, 128, H).transpose(1, 0, 2)).astype(bf)
        ow_h = np.ascontiguousarray(
            o_w[2 * tp:2 * tp + 2].reshape(2, 2, 128, D).transpose(2, 0, 1, 3)).astype(bf)
        in_maps.append({
            "xT": np.ascontiguousarray(x[b].T).astype(bf),
            "qw": qw_h,
            "kw": kw_h,
            "vw": vw_h,
            "ow": ow_h,
            "cosT": np.ascontiguousarray(np.cos(sinu).T).astype(np.float32),
            "sinT": np.ascontiguousarray(np.sin(sinu).T).astype(np.float32),
            "masks": masks_np,
            "scs": scs_np,
        })

    res = run_bass_kernel_spmd(nc, in_maps, core_ids=list(range(8)))
    last_results = res

    out = np.zeros((B, T, D), np.float32)
    for core in range(8):
        out[core // 4] += res.results[core]["out"].astype(np.float32)
    return out


# revision 10
# speedup vs baseline: 1.2639x; 1.0188x over previous
"""Sliding-window GQA attention (B=2,T=2048,D=2048,N=8,K=4,H=256,W=1024) on 8 trn2 cores.

Sharding: batch over 2 (fsdp) x heads over 4 (tp). Core (b, tp) computes 2 q heads /
1 kv head for batch b; partial [T, D] outputs are summed over tp on the host.

v2 (bf16): all matmul operands bf16 (fp32 PSUM accumulation) — same PE stream
rate as float32r but FWL halves weight-load time (phase B was LDW-bound), DMA
bytes halve, and SBUF pressure drops. Activation-table thrash eliminated:
phase A ACT = Square+Rsqrt only, phase B ACT = Exp only (1/den moved to the
DVE reciprocal_approx_fast custom op, output copies all on DVE). Host packs
weights in SBUF layout so each weight tensor is one batched DMA. Half-masked
edge key-blocks (j=i+1, j=i-8) only compute their valid 128-query half.

Per-core device pipeline:
  A: qT/kT = W^T x^T (head-dim on partitions) and v (natural layout), streaming
     x^T by 512-token quarters; fused RMS-norm + RoPE out of PSUM.
  B: per 256-token query pair: logits^T = kT^T qT per 128-key block (window
     blocks only), exp on ACT (no max-subtraction: |logit| <= ~6), triangular
     masks on DVE, denominator + P^T V via PE accumulation, divide via
     DVE approx-reciprocal.
  C: out = pvT^T o_w accumulated over local heads, emitted lagged one pair
     behind phase B so its matmuls fill exp-wait windows.
"""
import os

import numpy as np
import ml_dtypes

import concourse.bacc as bacc
import concourse.mybir as mybir
from concourse.tile import TileContext
from concourse.bass_utils import run_bass_kernel_spmd

try:  # pragma: no cover - profiling hook is optional
    from antenv.axon_hooks import get_axon_ntff_profile_hook  # noqa: F401
except ImportError:
    os.environ.setdefault("BASS_NEVER_TRACE", "1")


F32 = mybir.dt.float32
BF = mybir.dt.bfloat16
AF = mybir.ActivationFunctionType
OP = mybir.AluOpType

B, T, D = 2, 2048, 2048
N, KV, H = 8, 4, 256
WINDOW = 1024
BASE_FREQ = 10000.0
EPS = 1e-6
NB = T // 128          # 16 token blocks
NQ = 4                 # t quarters for projections (512 each)
NPAIR = 8              # query-block pairs (256 tokens each)


def _jlist(i):
    return list(range(max(0, i - 8), i + 2))


def _build():
    nc = bacc.Bacc(None)

    xT = nc.dram_tensor("xT", [128, 16, T], BF, kind="ExternalInput")
    qw = nc.dram_tensor("qw", [128, 2, 16, 256], BF, kind="ExternalInput")
    kw = nc.dram_tensor("kw", [128, 16, 256], BF, kind="ExternalInput")
    vw = nc.dram_tensor("vw", [128, 16, 256], BF, kind="ExternalInput")
    ow = nc.dram_tensor("ow", [128, 2, 2, D], BF, kind="ExternalInput")
    cosT = nc.dram_tensor("cosT", [128, T], F32, kind="ExternalInput")
    sinT = nc.dram_tensor("sinT", [128, T], F32, kind="ExternalInput")
    masks = nc.dram_tensor("masks", [128, 2, 128], BF, kind="ExternalInput")
    scs = nc.dram_tensor("scs", [128, 2, 2], F32, kind="ExternalInput")  # (1+scale)[q/k][hh]
    out = nc.dram_tensor("out", [T, D], BF, kind="ExternalOutput")

    with TileContext(nc) as tc:
        with tc.tile_pool(name="pers", bufs=1) as pers:
            kT_sb = pers.tile([128, 2, T], BF)
            v_sb = pers.tile([128, NB, H], BF)
            qT_sb = pers.tile([128, 2, 2, T], BF)
            kw_sb = pers.tile([128, 16, 256], BF)
            vw_sb = pers.tile([128, 16, 256], BF)
            qw_sb = pers.tile([128, 2, 16, 256], BF)
            cos_sb = pers.tile([128, T], F32)
            sin_sb = pers.tile([128, T], F32)
            scs_sb = pers.tile([128, 2, 2], F32)
            ones32 = pers.tile([128, 128], F32)
            ones = pers.tile([128, 128], BF)
            bias_q = pers.tile([128, 1], F32)
            bias_k = pers.tile([128, 1], F32)
            pvT_sb = pers.tile([128, 2, 2, T], BF)
            ow_sb = pers.tile([128, 2, 2, D], BF)
            masks_sb = pers.tile([128, 2, 128], BF)

            # batched weight DMAs ride the second HWDGE ring (ACT) so they
            # don't queue ahead of the x^T stream on the SP ring. Order by
            # first use: k proj, q proj, v proj, rope tables, phase B/C.
            nc.scalar.dma_start(out=kw_sb, in_=kw[:, :, :])
            nc.scalar.dma_start(out=qw_sb, in_=qw[:, :, :, :])
            nc.scalar.dma_start(out=vw_sb, in_=vw[:, :, :])
            nc.scalar.dma_start(out=cos_sb, in_=cosT[:, :])
            nc.scalar.dma_start(out=sin_sb, in_=sinT[:, :])
            nc.scalar.dma_start(out=scs_sb, in_=scs[:, :, :])
            nc.scalar.dma_start(out=masks_sb, in_=masks[:, :, :])
            nc.scalar.dma_start(out=ow_sb, in_=ow[:, :, :, :])
            nc.vector.memset(ones32, 1.0)
            nc.vector.tensor_copy(ones, ones32)
            nc.vector.memset(bias_q, float(H * EPS))
            nc.vector.memset(bias_k, EPS)

            # ---------------- Phase A: all projections + rms + rope ----------------
            with tc.tile_pool(name="xs", bufs=8) as xs, \
                 tc.tile_pool(name="ropep", bufs=1) as ropep, \
                 tc.tile_pool(name="psA", bufs=1, space="PSUM") as psum:

                def rope_emit(p0, p1, dst, kind, cs_t, ss_t):
                    # p0/p1: [128, 512] psum (raw proj h-halves); dst: [128, 2, 512] bf16 view
                    sq0 = ropep.tile([128, 512], BF, tag="sq0", bufs=2)
                    sq1 = ropep.tile([128, 512], BF, tag="sq1", bufs=2)
                    nc.scalar.activation(sq0, p0, AF.Square)
                    nc.scalar.activation(sq1, p1, AF.Square)
                    pss = psum.tile([128, 512], F32, tag="pss", bufs=1)
                    nc.tensor.matmul(pss, ones, sq0, start=True, stop=False)
                    nc.tensor.matmul(pss, ones, sq1, start=False, stop=True)
                    rs = ropep.tile([128, 512], F32, tag="rs", bufs=2)
                    if kind == "q":
                        # 1/16 * rsqrt(ss/256 + eps) == 1/sqrt(ss + 256*eps)
                        nc.scalar.activation(rs, pss, AF.Abs_reciprocal_sqrt,
                                             scale=1.0, bias=bias_q)
                    else:
                        nc.scalar.activation(rs, pss, AF.Abs_reciprocal_sqrt,
                                             scale=1.0 / H, bias=bias_k)
                    cs = ropep.tile([128, 512], F32, tag="cs", bufs=2)
                    ss = ropep.tile([128, 512], F32, tag="ss", bufs=2)
                    nc.vector.tensor_tensor(cs, cs_t, rs, OP.mult)
                    nc.vector.tensor_tensor(ss, ss_t, rs, OP.mult)
                    ki = 0 if kind == "q" else 1
                    s0 = scs_sb[:, ki, 0:1]
                    s1 = scs_sb[:, ki, 1:2]
                    t0 = ropep.tile([128, 512], F32, tag="t0", bufs=2)
                    t1 = ropep.tile([128, 512], F32, tag="t1", bufs=2)
                    nc.vector.scalar_tensor_tensor(t0, p0, s0, cs, OP.mult, OP.mult)
                    nc.vector.scalar_tensor_tensor(t1, p1, s1, ss, OP.mult, OP.mult)
                    nc.vector.tensor_tensor(dst[:, 0, :], t0, t1, OP.subtract)
                    t2 = ropep.tile([128, 512], F32, tag="t0", bufs=2)
                    t3 = ropep.tile([128, 512], F32, tag="t1", bufs=2)
                    nc.vector.scalar_tensor_tensor(t2, p1, s1, cs, OP.mult, OP.mult)
                    nc.vector.scalar_tensor_tensor(t3, p0, s0, ss, OP.mult, OP.mult)
                    nc.vector.tensor_tensor(dst[:, 1, :], t2, t3, OP.add)

                for qt in range(NQ):
                    tq = slice(512 * qt, 512 * (qt + 1))
                    # x^T quarter arrives as 4 batched DMAs of 4 d-chunks each
                    # (fewer DMA_DIRECT2D issues on the sync queue).
                    xqs = []
                    for g in range(4):
                        xq = xs.tile([128, 4, 512], BF, tag="xq")
                        nc.sync.dma_start(out=xq, in_=xT[:, slice(4 * g, 4 * g + 4), tq])
                        xqs.append(xq)
                    xts = [xqs[d // 4][:, d % 4, :] for d in range(16)]
                    cs_t = cos_sb[:, tq]
                    ss_t = sin_sb[:, tq]

                    # k h-halves interleaved per d-chunk (separate PSUM banks) so
                    # PE consumption keeps pace with the x^T DMA stream early on.
                    # NB: accumulation groups must NOT interleave within one bank.
                    pk = [psum.tile([128, 512], F32, tag="pq", bufs=6, name=f"pk{qt}_{hh}")
                          for hh in range(2)]
                    for d in range(16):
                        nc.tensor.matmul(pk[0], kw_sb[:, d, 0:128], xts[d],
                                         start=(d == 0), stop=(d == 15))
                        nc.tensor.matmul(pk[1], kw_sb[:, d, 128:256], xts[d],
                                         start=(d == 0), stop=(d == 15))
                    rope_emit(pk[0], pk[1], kT_sb[:, :, tq], "k", cs_t, ss_t)
                    # q heads
                    for nl in range(2):
                        ps = []
                        for hh in range(2):
                            p = psum.tile([128, 512], F32, tag="pq", bufs=6)
                            hs = slice(128 * hh, 128 * (hh + 1))
                            for d in range(16):
                                nc.tensor.matmul(p, qw_sb[:, nl, d, hs], xts[d],
                                                 start=(d == 0), stop=(d == 15))
                            ps.append(p)
                        rope_emit(ps[0], ps[1], qT_sb[:, nl, :, tq], "q", cs_t, ss_t)
                    # v (natural layout) last: its PSUM evacuation is a short DVE
                    # copy, so the psA pool frees quickly at the A->B boundary.
                    for half in range(2):
                        p = psum.tile([128, 2, H], F32, tag="pva", bufs=1, name=f"pv{qt}_{half}")
                        tc0 = 4 * qt + 2 * half
                        for sub in range(2):
                            tl = slice(128 * (2 * half + sub), 128 * (2 * half + sub) + 128)
                            for d in range(16):
                                nc.tensor.matmul(p[:, sub, :], xts[d][:, tl], vw_sb[:, d, :],
                                                 start=(d == 0), stop=(d == 15))
                        nc.vector.tensor_copy(v_sb[:, tc0:tc0 + 2, :], p)

            # ---------------- Phases B + C ----------------
            if True:
                with tc.tile_pool(name="expt", bufs=12) as expt, \
                     tc.tile_pool(name="bw", bufs=4) as bw, \
                     tc.tile_pool(name="oc", bufs=3) as oc, \
                     tc.tile_pool(name="psB", bufs=1, space="PSUM") as psumB:

                    def emit_logits_exp(pi):
                        i = 2 * pi
                        q_lo = slice(256 * pi, 256 * pi + 128)   # first query half
                        q_hi = slice(256 * pi + 128, 256 * (pi + 1))
                        q_all = slice(256 * pi, 256 * (pi + 1))
                        js = _jlist(i)
                        ets = {}
                        up = masks_sb[:, 0, :]
                        lo = masks_sb[:, 1, :]
                        for nl in range(2):
                            for k in range(0, len(js), 2):
                                jp = js[k:k + 2]
                                lp = psumB.tile([128, 2, 256], F32, tag="lp", bufs=4)
                                et = expt.tile([128, 2, 256], BF, tag="et")
                                for x2, j in enumerate(jp):
                                    sj = slice(128 * j, 128 * (j + 1))
                                    if j == i + 1:      # keys above all of q_lo
                                        lps, qsl = lp[:, x2, 128:256], q_hi
                                        ecols = slice(128, 256)
                                    elif j == i - 8:    # keys in-window only for q_lo
                                        lps, qsl = lp[:, x2, 0:128], q_lo
                                        ecols = slice(0, 128)
                                    else:
                                        lps, qsl = lp[:, x2, :], q_all
                                        ecols = slice(0, 256)
                                    nc.tensor.matmul(lps, kT_sb[:, 0, sj],
                                                     qT_sb[:, nl, 0, qsl],
                                                     start=True, stop=False)
                                    nc.tensor.matmul(lps, kT_sb[:, 1, sj],
                                                     qT_sb[:, nl, 1, qsl],
                                                     start=False, stop=True)
                                    ets[(nl, j)] = (et, x2, ecols)
                                # uncomputed edge halves hold stale psum; their
                                # exp lands in et cols no consumer ever reads.
                                nc.scalar.activation(et, lp, AF.Exp)
                                for x2, j in enumerate(jp):
                                    if j == i + 1:
                                        nc.vector.tensor_tensor(
                                            et[:, x2, 128:256], et[:, x2, 128:256], up, OP.mult)
                                    elif j == i:
                                        nc.vector.tensor_tensor(
                                            et[:, x2, 0:128], et[:, x2, 0:128], up, OP.mult)
                                    elif j == i - 7:
                                        nc.vector.tensor_tensor(
                                            et[:, x2, 128:256], et[:, x2, 128:256], lo, OP.mult)
                                    elif j == i - 8:
                                        nc.vector.tensor_tensor(
                                            et[:, x2, 0:128], et[:, x2, 0:128], lo, OP.mult)
                        return ets

                    def emit_tail(pi, ets):
                        i = 2 * pi
                        tqs = slice(256 * pi, 256 * (pi + 1))
                        js = _jlist(i)
                        for nl in range(2):
                            pd = psumB.tile([128, 256], F32, tag="pd", bufs=1)
                            for idx, j in enumerate(js):
                                et, x2, ecols = ets[(nl, j)]
                                nc.tensor.matmul(pd[:, ecols], ones, et[:, x2, ecols],
                                                 start=(idx == 0), stop=(idx == len(js) - 1))
                            rc = bw.tile([128, 256], F32, tag="rc")
                            nc.vector.reciprocal_approx_fast(rc, pd)
                            for hh in range(2):
                                pv = psumB.tile([128, 256], F32, tag="pvb", bufs=3)
                                hs = slice(128 * hh, 128 * (hh + 1))
                                for idx, j in enumerate(js):
                                    et, x2, ecols = ets[(nl, j)]
                                    nc.tensor.matmul(pv[:, ecols], v_sb[:, j, hs],
                                                     et[:, x2, ecols],
                                                     start=(idx == 0), stop=(idx == len(js) - 1))
                                nc.vector.tensor_tensor(pvT_sb[:, nl, hh, tqs], pv, rc, OP.mult)

                    def emit_oproj(pi):
                        # output projection for this pair's two token blocks;
                        # one batched out-DMA per token block.
                        for tb in (2 * pi, 2 * pi + 1):
                            ts_ = slice(128 * tb, 128 * (tb + 1))
                            od = oc.tile([128, 4, 512], BF, tag="od", bufs=3)
                            for dt in range(4):
                                dsl = slice(512 * dt, 512 * (dt + 1))
                                po = psumB.tile([128, 512], F32, tag="pvb", bufs=3)
                                step = 0
                                for nl in range(2):
                                    for hh in range(2):
                                        nc.tensor.matmul(po, pvT_sb[:, nl, hh, ts_],
                                                         ow_sb[:, nl, hh, dsl],
                                                         start=(step == 0), stop=(step == 3))
                                        step += 1
                                nc.vector.tensor_copy(od[:, dt, :], po)
                            nc.sync.dma_start(out=out[ts_, :], in_=od)

                    # o-proj lags one pair behind: its matmuls fill exp-wait
                    # windows and give the ow DMA time to land after phase A.
                    for pi in range(NPAIR):
                        emit_tail(pi, emit_logits_exp(pi))
                        if pi > 0:
                            emit_oproj(pi - 1)
                    emit_oproj(NPAIR - 1)

    nc.compile()
    return nc


_prog = None
last_results = None


def kernel(x, positions, q_w, k_w, v_w, o_w, q_norm_scale, k_norm_scale):
    global _prog, last_results
    x = np.asarray(x); positions = np.asarray(positions)
    q_w = np.asarray(q_w); k_w = np.asarray(k_w); v_w = np.asarray(v_w); o_w = np.asarray(o_w)
    q_norm_scale = np.asarray(q_norm_scale); k_norm_scale = np.asarray(k_norm_scale)

    if _prog is None:
        _prog = _build()
    nc = _prog

    bf = ml_dtypes.bfloat16

    # host-side constants
    j = np.arange(H // 2, dtype=np.float32)
    timescale = (BASE_FREQ ** (2.0 / H * j)).astype(np.float32)

    c = np.arange(128)[:, None]   # key within block (partition)
    r = np.arange(128)[None, :]   # query within block (column)
    up = (c <= r).astype(np.float32)
    lo = (c > r).astype(np.float32)
    masks_np = np.stack([up, lo], axis=1).astype(bf)  # [128, 2, 128]

    scs_np = np.empty((128, 2, 2), np.float32)
    scs_np[:, 0, 0] = 1.0 + q_norm_scale[:128]
    scs_np[:, 0, 1] = 1.0 + q_norm_scale[128:]
    scs_np[:, 1, 0] = 1.0 + k_norm_scale[:128]
    scs_np[:, 1, 1] = 1.0 + k_norm_scale[128:]

    in_maps = []
    for core in range(8):
        b, tp = core // 4, core % 4
        sinu = positions[b].astype(np.float32)[:, None] / timescale[None, :]  # [T, 128]
        qw_h = np.ascontiguousarray(
            q_w[2 * tp:2 * tp + 2].reshape(2, 16, 128, H).transpose(2, 0, 1, 3)).astype(bf)
        kw_h = np.ascontiguousarray(
            k_w[tp].reshape(16, 128, H).transpose(1, 0, 2)).astype(bf)
        vw_h = np.ascontiguousarray(
            v_w[tp].reshape(16, 128, H).transpose(1, 0, 2)).astype(bf)
        ow_h = np.ascontiguousarray(
            o_w[2 * tp:2 * tp + 2].reshape(2, 2, 128, D).transpose(2, 0, 1, 3)).astype(bf)
        # x^T pre-swizzled to SBUF layout [p, d_chunk, t]
        xT_h = np.ascontiguousarray(
            x[b].T.reshape(16, 128, T).transpose(1, 0, 2)).astype(bf)
        in_maps.append({
            "xT": xT_h,
            "qw": qw_h,
            "kw": kw_h,
            "vw": vw_h,
            "ow": ow_h,
            "cosT": np.ascontiguousarray(np.cos(sinu).T).astype(np.float32),
            "sinT": np.ascontiguousarray(np.sin(sinu).T).astype(np.float32),
            "masks": masks_np,
            "scs": scs_np,
        })

    res = run_bass_kernel_spmd(nc, in_maps, core_ids=list(range(8)))
    last_results = res

    out = np.zeros((B, T, D), np.float32)
    for core in range(8):
        out[core // 4] += res.results[core]["out"].astype(np.float32)
    return out
